# revision 27
# baseline (speedup 1.0000x reference)
"""Multi-head causal attention on 8 Trainium2 cores.

Reference model:
    xq = x + pos_embed
    q = xq @ W_Q^T, k = xq @ W_K^T (per head), v = x @ W_V^T
    out = sum_heads causal_softmax(q k^T / 8) @ v @ W_O^T

Sharding: 8 cores = 4 batches x 2 head-groups (8 heads each); host sums
the two head-group partials per batch (the "all-reduce").

Per-core dataflow (all matmuls float32r = full-rate fp32 storage):
  A. transpose W_Q/W_K/W_V on PE -> wT [m, ih]
  B. x/pos tiles -> add -> PE-transpose -> xqT/xT [m, seq] blocks ->
     QT/KT [ih, seq] (head pairs pack one 128-row chunk), V [seq, i, h|1]
     (ones column yields softmax normalizers for free)
  C. per head: scoresT [k, q] = KT-rows.T @ QT-rows -> exp on ACT
     (scale=1/8) -> causal zeroing via gpsimd affine_select on diagonal
     tiles -> zT[0:65] += V_aug.T @ expS (row 64 = sum Z) ->
     recip(Z) -> gpsimd partition_broadcast -> DVE mult; odd heads are
     shifted to partitions 64..127 via SBUF->SBUF DMA.
     W_O transposed here too (same DMA-shift pairing).
  D. out[q, m] += zTf-chunk.T @ woT-chunk over 4 head-pair chunks.
"""

import sys

if "/opt/trn_rl_repo" not in sys.path:
    sys.path.insert(0, "/opt/trn_rl_repo")

import numpy as np

SEQ = 2048
DM = 1024
NH = 8          # heads per core
DH = 64
IH = NH * DH    # 512
MC = DM // 128  # 8 m-chunks
ST = SEQ // 128  # 16 seq tiles
NQB = SEQ // 512  # 4 query blocks
GRP = 2         # key tiles per exp group (2 psum banks)

_BUILT = None


def _build():
    import concourse.mybir as mybir
    import concourse.tile as tile
    from concourse import bacc
    from concourse.masks import make_identity

    dt = mybir.dt
    f32, f32r = dt.float32, dt.float32r
    AF = mybir.ActivationFunctionType
    Alu = mybir.AluOpType

    nc = bacc.Bacc("TRN2", target_bir_lowering=False, debug=False)
    x_d = nc.dram_tensor("x_s", [SEQ, DM], f32, kind="ExternalInput")
    pos_d = nc.dram_tensor("pos_s", [SEQ, DM], f32, kind="ExternalInput")
    wq_d = nc.dram_tensor("wq_s", [NH, DH, DM], f32, kind="ExternalInput")
    wk_d = nc.dram_tensor("wk_s", [NH, DH, DM], f32, kind="ExternalInput")
    wv_d = nc.dram_tensor("wv_s", [NH, DH, DM], f32, kind="ExternalInput")
    wo_d = nc.dram_tensor("wo_s", [NH, DM, DH], f32, kind="ExternalInput")
    out_d = nc.dram_tensor("out_s", [SEQ, DM], f32, kind="ExternalOutput")

    with tile.TileContext(nc) as tc:
        with tc.tile_pool(name="const", bufs=1) as cp, \
             tc.tile_pool(name="qkv", bufs=1) as qkvp:
            ident = cp.tile([128, 128], f32)
            make_identity(nc, ident[:])
            ones_st = cp.tile([128, 1], f32)
            nc.gpsimd.memset(ones_st[:], 1.0)

            QT = qkvp.tile([128, IH // 128, SEQ], f32r)  # [ih_in, chunk, seq]
            KT = qkvp.tile([128, IH // 128, SEQ], f32r)
            # [seq_in, seq_tile, i*(h|1) + 63 pad] — pad lets the PV matmul use
            # a full 128-col stationary operand (fp32r is half-rate below 128)
            V = qkvp.tile([128, ST, NH * (DH + 1) + 63], f32r)

            # zero V's pad region so the padded PV stationary reads never
            # see NaN garbage (copy from a zeroed f32 staging tile)
            zero_st = cp.tile([128, 1], f32)
            nc.gpsimd.memset(zero_st[:], 0.0)
            nc.vector.tensor_copy(
                V[:, :, NH * (DH + 1):],
                zero_st[:, 0:1].to_broadcast([128, ST, 63]))

            # ---------------- Phase A: q/k/v weight transposes -------------
            with tc.tile_pool(name="wts", bufs=1) as wp:
                wqT = wp.tile([128, MC, IH], f32r)   # [m_in, m_chunk, ih]
                wkT = wp.tile([128, MC, IH], f32r)
                wvT = wp.tile([128, MC, IH], f32r)
                with tc.tile_pool(name="wnat", bufs=2) as wnat, \
                     tc.tile_pool(name="ppsA", bufs=4, space="PSUM") as ppsA:
                    for w_d, wT in ((wq_d, wqT), (wk_d, wkT), (wv_d, wvT)):
                        w_flat = w_d.ap().rearrange("i h m -> (i h) m")
                        for c in range(IH // 128):
                            wn = wnat.tile([128, DM], f32, tag="wnat", name="wn")
                            nc.sync.dma_start(wn[:], w_flat[c * 128:(c + 1) * 128, :])
                            for g in range(2):
                                ps = ppsA.tile([128, 512], f32, tag="tp", name="psA")
                                for j in range(4):
                                    mc = g * 4 + j
                                    nc.tensor.transpose(
                                        ps[:, j * 128:(j + 1) * 128],
                                        wn[:, mc * 128:(mc + 1) * 128], ident[:])
                                nc.vector.tensor_copy(
                                    wT[:, g * 4:(g + 1) * 4, c * 128:(c + 1) * 128],
                                    ps[:].rearrange("p (a b) -> p a b", a=4))

                # ------------ Phase B: x transposes + Q/K/V projections ----
                with tc.tile_pool(name="xnat", bufs=2) as xnat, \
                     tc.tile_pool(name="xtr", bufs=1) as xtr, \
                     tc.tile_pool(name="ppsB", bufs=4, space="PSUM") as ppsB:
                    for sb in range(SEQ // 512):
                        xqT_blk = xtr.tile([128, MC, 512], f32r, tag="xqT",
                                           name="xqT_blk")
                        xT_blk = xtr.tile([128, MC, 512], f32r, tag="xT",
                                          name="xT_blk")
                        for stl in range(4):
                            st = sb * 4 + stl
                            x_nat = xnat.tile([128, DM], f32, tag="x", name="x_nat")
                            nc.sync.dma_start(
                                x_nat[:], x_d.ap()[st * 128:(st + 1) * 128, :])
                            pos_nat = xnat.tile([128, DM], f32, tag="pos",
                                                name="pos_nat")
                            nc.sync.dma_start(
                                pos_nat[:], pos_d.ap()[st * 128:(st + 1) * 128, :])
                            # xq = x + pos (pos tile is dead after this)
                            nc.vector.tensor_add(pos_nat[:], x_nat[:], pos_nat[:])
                            for src, dst in ((pos_nat, xqT_blk), (x_nat, xT_blk)):
                                for g in range(2):
                                    ps = ppsB.tile([128, 512], f32, tag="tp",
                                                   name="psB")
                                    for j in range(4):
                                        mc = g * 4 + j
                                        nc.tensor.transpose(
                                            ps[:, j * 128:(j + 1) * 128],
                                            src[:, mc * 128:(mc + 1) * 128],
                                            ident[:])
                                    nc.vector.tensor_copy(
                                        dst[:, g * 4:(g + 1) * 4,
                                            stl * 128:(stl + 1) * 128],
                                        ps[:].rearrange("p (a b) -> p a b", a=4))
                        for wT, dstT in ((wqT, QT), (wkT, KT)):
                            for c in range(IH // 128):
                                ps = ppsB.tile([128, 512], f32, tag="tp",
                                               name="ps_qk")
                                for mc in range(MC):
                                    nc.tensor.matmul(
                                        ps[:], wT[:, mc, c * 128:(c + 1) * 128],
                                        xqT_blk[:, mc, :],
                                        start=(mc == 0), stop=(mc == MC - 1))
                                nc.vector.tensor_copy(
                                    dstT[:, c, sb * 512:(sb + 1) * 512], ps[:])
                        for stl in range(4):
                            st = sb * 4 + stl
                            ps = ppsB.tile([128, 512], f32, tag="tp", name="ps_v")
                            for mc in range(MC):
                                nc.tensor.matmul(
                                    ps, xT_blk[:, mc, stl * 128:(stl + 1) * 128],
                                    wvT[:, mc, :],
                                    start=(mc == 0), stop=(mc == MC - 1))
                            nc.vector.tensor_copy(
                                V[:, st, 0:NH * (DH + 1)].rearrange(
                                    "p (i x) -> p i x", i=NH)[:, :, 0:DH],
                                ps[:].rearrange("p (i h) -> p i h", i=NH))
                    nc.vector.tensor_copy(
                        V[:, :, 0:NH * (DH + 1)].rearrange(
                            "p s (i x) -> p s i x", i=NH)[:, :, :, DH:DH + 1],
                        ones_st[:, 0:1].to_broadcast([128, ST, NH, 1]))

            # ---------------- Phase C: attention (+ W_O transposes) --------
            with tc.tile_pool(name="zwo", bufs=1, side="right") as zwop, \
                 tc.tile_pool(name="apsum", bufs=4, space="PSUM") as apsum:
                zTf = zwop.tile([128, NH // 2, SEQ], f32r)  # [h-pair, chunk, q]
                woT = zwop.tile([128, NH // 2, DM], f32r)   # [h-pair, chunk, m]

                with tc.tile_pool(name="wonat", bufs=1) as wonat, \
                     tc.tile_pool(name="expp", bufs=3) as expp, \
                     tc.tile_pool(name="small", bufs=2) as small, \
                     tc.tile_pool(name="spsum", bufs=2, space="PSUM") as spsum:
                    # W_O [i, m, h] -> woT [h(pair), c, m] via PE transpose;
                    # odd heads partition-shifted by SBUF->SBUF DMA.
                    wo_nat = wonat.tile([128, NH, 8, DH], f32, name="wo_nat")
                    nc.sync.dma_start(
                        wo_nat[:],
                        wo_d.ap().rearrange("i (mo mi) h -> mi i mo h", mi=128))
                    for c in range(NH // 2):
                        for g in range(2):
                            for hh in range(2):
                                i = 2 * c + hh
                                ps = apsum.tile([128, 512], f32, tag="acc",
                                                name="ps_wo")
                                for j in range(4):
                                    mo = g * 4 + j
                                    nc.tensor.transpose(
                                        ps[0:64, j * 128:(j + 1) * 128],
                                        wo_nat[:, i, mo, :], ident[:])
                                if hh == 0:
                                    nc.vector.tensor_copy(
                                        woT[0:64, c, g * 512:(g + 1) * 512]
                                        .rearrange("p (a b) -> p a b", a=4),
                                        ps[0:64].rearrange("p (a b) -> p a b", a=4))
                                else:
                                    stw = small.tile([64, 512], f32r, tag="stg",
                                                     name="stw")
                                    nc.vector.tensor_copy(stw[:], ps[0:64])
                                    nc.sync.dma_start(
                                        woT[64:128, c, g * 512:(g + 1) * 512],
                                        stw[:])

                    def make_norm(c, qb, zps):
                        def emit_norm():
                            for hh in range(2):
                                recip = small.tile([1, 512], f32, tag="recip",
                                                   name="recip")
                                nc.vector.reciprocal(recip[:], zps[hh][64:65, :])
                                bc = small.tile([64, 512], f32, tag="bc",
                                                name="bc")
                                nc.gpsimd.partition_broadcast(bc[:], recip[:])
                                if hh == 0:
                                    nc.vector.tensor_mul(
                                        zTf[0:64, c, qb * 512:(qb + 1) * 512],
                                        zps[hh][0:64, :], bc[:])
                                else:
                                    stg = small.tile([64, 512], f32r, tag="stg",
                                                     name="stg")
                                    nc.vector.tensor_mul(stg[:], zps[hh][0:64, :],
                                                         bc[:])
                                    nc.sync.dma_start(
                                        zTf[64:128, c, qb * 512:(qb + 1) * 512],
                                        stg[:])
                        return emit_norm

                    pending_norm = None
                    for c in range(NH // 2):
                        for qb in range(NQB):
                            nkt = 4 * qb + 4
                            zps = [apsum.tile([128, 512], f32, tag="acc",
                                              name=f"z{hh}") for hh in range(2)]
                            for g0 in range(0, nkt, GRP):
                                kts = list(range(g0, min(g0 + GRP, nkt)))
                                exs = []
                                for hh in range(2):
                                    r0 = hh * 64
                                    sc = spsum.tile([128, GRP * 512], f32,
                                                    tag="sc", name="sc")
                                    for jj, kt in enumerate(kts):
                                        nc.tensor.matmul(
                                            sc[:, jj * 512:(jj + 1) * 512],
                                            KT[r0:r0 + 64, c,
                                               kt * 128:(kt + 1) * 128],
                                            QT[r0:r0 + 64, c,
                                               qb * 512:(qb + 1) * 512],
                                            start=True, stop=True,
                                            tile_position=(r0, 0))
                                    ex = expp.tile([128, GRP * 512], f32r,
                                                   tag="ex", name="ex")
                                    n = len(kts) * 512
                                    nc.scalar.activation(ex[:, :n], sc[:, :n],
                                                         AF.Exp, scale=0.125)
                                    exs.append(ex)
                                if pending_norm is not None:
                                    pending_norm()
                                    pending_norm = None
                                for hh in range(2):
                                    i = 2 * c + hh
                                    ex = exs[hh]
                                    for jj, kt in enumerate(kts):
                                        if kt >= 4 * qb:  # diagonal: causal zero
                                            nc.gpsimd.affine_select(
                                                out=ex[:, jj * 512:(jj + 1) * 512],
                                                in_=ex[:, jj * 512:(jj + 1) * 512],
                                                compare_op=Alu.is_ge,
                                                fill=0.0,
                                                base=512 * qb - 128 * kt,
                                                pattern=[[1, 512]],
                                                channel_multiplier=-1)
                                        nc.tensor.matmul(
                                            zps[hh][:],
                                            V[:, kt, i * (DH + 1):
                                              i * (DH + 1) + 128],
                                            ex[:, jj * 512:(jj + 1) * 512],
                                            start=(kt == 0), stop=(kt == nkt - 1))
                            pending_norm = make_norm(c, qb, zps)
                    if pending_norm is not None:
                        pending_norm()
                        pending_norm = None

                # ------------ Phase D: output projection -------------------
                with tc.tile_pool(name="outsb", bufs=2) as outsb:
                    for qt in range(ST):
                        osb = outsb.tile([128, DM], f32, tag="osb", name="osb")
                        for mb in range(2):
                            po = apsum.tile([128, 512], f32, tag="acc", name="po")
                            for c in range(NH // 2):
                                nc.tensor.matmul(
                                    po, zTf[:, c, qt * 128:(qt + 1) * 128],
                                    woT[:, c, mb * 512:(mb + 1) * 512],
                                    start=(c == 0), stop=(c == NH // 2 - 1))
                            nc.vector.tensor_copy(osb[:, mb * 512:(mb + 1) * 512],
                                                  po)
                        nc.sync.dma_start(out_d.ap()[qt * 128:(qt + 1) * 128, :],
                                          osb[:])

    nc.compile()
    return nc


def _get_nc():
    global _BUILT
    if _BUILT is None:
        _BUILT = _build()
    return _BUILT


def run(inputs, trace=False):
    from concourse import bass_utils

    nc = _get_nc()
    x = np.ascontiguousarray(inputs["x"], dtype=np.float32)
    pos = np.ascontiguousarray(inputs["pos_embed"], dtype=np.float32)
    wq, wk, wv, wo = (np.asarray(inputs[k], dtype=np.float32)
                      for k in ("W_Q", "W_K", "W_V", "W_O"))
    in_maps = []
    for core in range(8):
        b, g = core // 2, core % 2
        hs = slice(g * NH, (g + 1) * NH)
        in_maps.append({
            "x_s": np.ascontiguousarray(x[b]),
            "pos_s": np.ascontiguousarray(pos[b]),
            "wq_s": np.ascontiguousarray(wq[hs]),
            "wk_s": np.ascontiguousarray(wk[hs]),
            "wv_s": np.ascontiguousarray(wv[hs]),
            "wo_s": np.ascontiguousarray(wo[hs]),
        })
    res = bass_utils.run_bass_kernel_spmd(
        nc, in_maps, core_ids=list(range(8)), trace=trace)
    out = np.empty((4, SEQ, DM), dtype=np.float32)
    for b in range(4):
        out[b] = res.results[2 * b]["out_s"] + res.results[2 * b + 1]["out_s"]
    return out, res.exec_time_ns


def kernel(**inputs):
    out, _ = run(inputs, trace=False)
    return out


# revision 28
# speedup vs baseline: 1.1615x; 1.1615x over previous
"""Multi-head causal attention on 8 Trainium2 cores.

Reference model:
    xq = x + pos_embed
    q = xq @ W_Q^T, k = xq @ W_K^T (per head), v = x @ W_V^T
    out = sum_heads causal_softmax(q k^T / 8) @ v @ W_O^T

Sharding: 8 cores = 4 batches x 2 head-groups (8 heads each); host sums
the two head-group partials per batch (the "all-reduce").

Per-core dataflow (all matmuls float32r = full-rate fp32 storage):
  A. transpose W_Q/W_K/W_V on PE -> wT [m, ih]
  B. x/pos tiles -> add -> PE-transpose -> xqT/xT [m, seq] blocks ->
     QT/KT [ih, seq] (head pairs pack one 128-row chunk), V [seq, i, h|1]
     (ones column yields softmax normalizers for free)
  C. per head: scoresT [k, q] = KT-rows.T @ QT-rows -> exp on ACT
     (scale=1/8) -> causal zeroing via gpsimd affine_select on diagonal
     tiles -> zT[0:65] += V_aug.T @ expS (row 64 = sum Z) ->
     recip(Z) -> gpsimd partition_broadcast -> DVE mult; odd heads are
     shifted to partitions 64..127 via SBUF->SBUF DMA.
     W_O transposed here too (same DMA-shift pairing).
  D. out[q, m] += zTf-chunk.T @ woT-chunk over 4 head-pair chunks.
"""

import sys

if "/opt/trn_rl_repo" not in sys.path:
    sys.path.insert(0, "/opt/trn_rl_repo")

import numpy as np

SEQ = 2048
DM = 1024
NH = 8          # heads per core
DH = 64
IH = NH * DH    # 512
MC = DM // 128  # 8 m-chunks
ST = SEQ // 128  # 16 seq tiles
NQB = SEQ // 512  # 4 query blocks
GRP = 2         # key tiles per exp group (2 psum banks)

_BUILT = None


def _build():
    import concourse.mybir as mybir
    import concourse.tile as tile
    from concourse import bacc
    from concourse.masks import make_identity

    dt = mybir.dt
    f32, f32r, bf16 = dt.float32, dt.float32r, dt.bfloat16
    AF = mybir.ActivationFunctionType
    Alu = mybir.AluOpType

    nc = bacc.Bacc("TRN2", target_bir_lowering=False, debug=False)
    x_d = nc.dram_tensor("x_s", [SEQ, DM], f32, kind="ExternalInput")
    pos_d = nc.dram_tensor("pos_s", [SEQ, DM], f32, kind="ExternalInput")
    wq_d = nc.dram_tensor("wq_s", [NH, DH, DM], f32, kind="ExternalInput")
    wk_d = nc.dram_tensor("wk_s", [NH, DH, DM], f32, kind="ExternalInput")
    wv_d = nc.dram_tensor("wv_s", [NH, DH, DM], f32, kind="ExternalInput")
    wo_d = nc.dram_tensor("wo_s", [NH, DM, DH], f32, kind="ExternalInput")
    out_d = nc.dram_tensor("out_s", [SEQ, DM], f32, kind="ExternalOutput")

    with tile.TileContext(nc) as tc:
        with tc.tile_pool(name="const", bufs=1) as cp, \
             tc.tile_pool(name="qkv", bufs=1) as qkvp:
            ident = cp.tile([128, 128], f32)
            make_identity(nc, ident[:])
            ones_st = cp.tile([128, 1], f32)
            nc.gpsimd.memset(ones_st[:], 1.0)

            QT = qkvp.tile([128, IH // 128, SEQ], bf16)  # [ih_in, chunk, seq]
            KT = qkvp.tile([128, IH // 128, SEQ], bf16)
            # [seq_in, seq_tile, i*(h|1) + 63 pad] — pad lets the PV matmul use
            # a full 128-col stationary operand
            V = qkvp.tile([128, ST, NH * (DH + 1) + 63], bf16)

            # zero V's pad region so the padded PV stationary reads never
            # see NaN garbage (copy from a zeroed f32 staging tile)
            zero_st = cp.tile([128, 1], f32)
            nc.gpsimd.memset(zero_st[:], 0.0)
            nc.vector.tensor_copy(
                V[:, :, NH * (DH + 1):],
                zero_st[:, 0:1].to_broadcast([128, ST, 63]))

            # ---------------- Phase A: q/k/v weight transposes -------------
            with tc.tile_pool(name="wts", bufs=1) as wp:
                wqT = wp.tile([128, MC, IH], f32r)   # [m_in, m_chunk, ih]
                wkT = wp.tile([128, MC, IH], f32r)
                wvT = wp.tile([128, MC, IH], f32r)
                with tc.tile_pool(name="wnat", bufs=2) as wnat, \
                     tc.tile_pool(name="ppsA", bufs=4, space="PSUM") as ppsA:
                    for w_d, wT in ((wq_d, wqT), (wk_d, wkT), (wv_d, wvT)):
                        w_flat = w_d.ap().rearrange("i h m -> (i h) m")
                        for c in range(IH // 128):
                            wn = wnat.tile([128, DM], f32, tag="wnat", name="wn")
                            nc.sync.dma_start(wn[:], w_flat[c * 128:(c + 1) * 128, :])
                            for g in range(2):
                                ps = ppsA.tile([128, 512], f32, tag="tp", name="psA")
                                for j in range(4):
                                    mc = g * 4 + j
                                    nc.tensor.transpose(
                                        ps[:, j * 128:(j + 1) * 128],
                                        wn[:, mc * 128:(mc + 1) * 128], ident[:])
                                nc.vector.tensor_copy(
                                    wT[:, g * 4:(g + 1) * 4, c * 128:(c + 1) * 128],
                                    ps[:].rearrange("p (a b) -> p a b", a=4))

                # ------------ Phase B: x transposes + Q/K/V projections ----
                with tc.tile_pool(name="xnat", bufs=2) as xnat, \
                     tc.tile_pool(name="xtr", bufs=1) as xtr, \
                     tc.tile_pool(name="ppsB", bufs=4, space="PSUM") as ppsB:
                    for sb in range(SEQ // 512):
                        xqT_blk = xtr.tile([128, MC, 512], f32r, tag="xqT",
                                           name="xqT_blk")
                        xT_blk = xtr.tile([128, MC, 512], f32r, tag="xT",
                                          name="xT_blk")
                        for stl in range(4):
                            st = sb * 4 + stl
                            x_nat = xnat.tile([128, DM], f32, tag="x", name="x_nat")
                            nc.sync.dma_start(
                                x_nat[:], x_d.ap()[st * 128:(st + 1) * 128, :])
                            pos_nat = xnat.tile([128, DM], f32, tag="pos",
                                                name="pos_nat")
                            nc.sync.dma_start(
                                pos_nat[:], pos_d.ap()[st * 128:(st + 1) * 128, :])
                            # xq = x + pos (pos tile is dead after this)
                            nc.vector.tensor_add(pos_nat[:], x_nat[:], pos_nat[:])
                            for src, dst in ((pos_nat, xqT_blk), (x_nat, xT_blk)):
                                for g in range(2):
                                    ps = ppsB.tile([128, 512], f32, tag="tp",
                                                   name="psB")
                                    for j in range(4):
                                        mc = g * 4 + j
                                        nc.tensor.transpose(
                                            ps[:, j * 128:(j + 1) * 128],
                                            src[:, mc * 128:(mc + 1) * 128],
                                            ident[:])
                                    nc.vector.tensor_copy(
                                        dst[:, g * 4:(g + 1) * 4,
                                            stl * 128:(stl + 1) * 128],
                                        ps[:].rearrange("p (a b) -> p a b", a=4))
                        for wT, dstT in ((wqT, QT), (wkT, KT)):
                            for c in range(IH // 128):
                                ps = ppsB.tile([128, 512], f32, tag="tp",
                                               name="ps_qk")
                                for mc in range(MC):
                                    nc.tensor.matmul(
                                        ps[:], wT[:, mc, c * 128:(c + 1) * 128],
                                        xqT_blk[:, mc, :],
                                        start=(mc == 0), stop=(mc == MC - 1))
                                nc.vector.tensor_copy(
                                    dstT[:, c, sb * 512:(sb + 1) * 512], ps[:])
                        for stl in range(4):
                            st = sb * 4 + stl
                            ps = ppsB.tile([128, 512], f32, tag="tp", name="ps_v")
                            for mc in range(MC):
                                nc.tensor.matmul(
                                    ps, xT_blk[:, mc, stl * 128:(stl + 1) * 128],
                                    wvT[:, mc, :],
                                    start=(mc == 0), stop=(mc == MC - 1))
                            nc.vector.tensor_copy(
                                V[:, st, 0:NH * (DH + 1)].rearrange(
                                    "p (i x) -> p i x", i=NH)[:, :, 0:DH],
                                ps[:].rearrange("p (i h) -> p i h", i=NH))
                    nc.vector.tensor_copy(
                        V[:, :, 0:NH * (DH + 1)].rearrange(
                            "p s (i x) -> p s i x", i=NH)[:, :, :, DH:DH + 1],
                        ones_st[:, 0:1].to_broadcast([128, ST, NH, 1]))

            # ---------------- Phase C: attention (+ W_O transposes) --------
            with tc.tile_pool(name="zwo", bufs=1, side="right") as zwop, \
                 tc.tile_pool(name="apsum", bufs=4, space="PSUM") as apsum:
                zTf = zwop.tile([128, NH // 2, SEQ], f32r)  # [h-pair, chunk, q]
                woT = zwop.tile([128, NH // 2, DM], f32r)   # [h-pair, chunk, m]

                with tc.tile_pool(name="wonat", bufs=1) as wonat, \
                     tc.tile_pool(name="expp", bufs=3) as expp, \
                     tc.tile_pool(name="small", bufs=2) as small, \
                     tc.tile_pool(name="spsum", bufs=2, space="PSUM") as spsum:
                    # W_O [i, m, h] -> woT [h(pair), c, m] via PE transpose;
                    # odd heads partition-shifted by SBUF->SBUF DMA.
                    wo_nat = wonat.tile([128, NH, 8, DH], f32, name="wo_nat")
                    nc.sync.dma_start(
                        wo_nat[:],
                        wo_d.ap().rearrange("i (mo mi) h -> mi i mo h", mi=128))
                    for c in range(NH // 2):
                        for g in range(2):
                            for hh in range(2):
                                i = 2 * c + hh
                                ps = apsum.tile([128, 512], f32, tag="acc",
                                                name="ps_wo")
                                for j in range(4):
                                    mo = g * 4 + j
                                    nc.tensor.transpose(
                                        ps[0:64, j * 128:(j + 1) * 128],
                                        wo_nat[:, i, mo, :], ident[:])
                                if hh == 0:
                                    nc.vector.tensor_copy(
                                        woT[0:64, c, g * 512:(g + 1) * 512]
                                        .rearrange("p (a b) -> p a b", a=4),
                                        ps[0:64].rearrange("p (a b) -> p a b", a=4))
                                else:
                                    stw = small.tile([64, 512], f32r, tag="stg",
                                                     name="stw")
                                    nc.vector.tensor_copy(stw[:], ps[0:64])
                                    nc.sync.dma_start(
                                        woT[64:128, c, g * 512:(g + 1) * 512],
                                        stw[:])

                    def make_norm(c, qb, zps):
                        def emit_norm():
                            for hh in range(2):
                                recip = small.tile([1, 512], f32, tag="recip",
                                                   name="recip")
                                nc.vector.reciprocal(recip[:], zps[hh][64:65, :])
                                bc = small.tile([64, 512], f32, tag="bc",
                                                name="bc")
                                nc.gpsimd.partition_broadcast(bc[:], recip[:])
                                if hh == 0:
                                    nc.vector.tensor_mul(
                                        zTf[0:64, c, qb * 512:(qb + 1) * 512],
                                        zps[hh][0:64, :], bc[:])
                                else:
                                    stg = small.tile([64, 512], f32r, tag="stg",
                                                     name="stg")
                                    nc.vector.tensor_mul(stg[:], zps[hh][0:64, :],
                                                         bc[:])
                                    nc.sync.dma_start(
                                        zTf[64:128, c, qb * 512:(qb + 1) * 512],
                                        stg[:])
                        return emit_norm

                    pending_norm = None
                    for c in range(NH // 2):
                        for qb in range(NQB):
                            nkt = 4 * qb + 4
                            zps = [apsum.tile([128, 512], f32, tag="acc",
                                              name=f"z{hh}") for hh in range(2)]
                            for g0 in range(0, nkt, GRP):
                                kts = list(range(g0, min(g0 + GRP, nkt)))
                                exs = []
                                for hh in range(2):
                                    r0 = hh * 64
                                    sc = spsum.tile([128, GRP * 512], f32,
                                                    tag="sc", name="sc")
                                    for jj, kt in enumerate(kts):
                                        nc.tensor.matmul(
                                            sc[:, jj * 512:(jj + 1) * 512],
                                            KT[r0:r0 + 64, c,
                                               kt * 128:(kt + 1) * 128],
                                            QT[r0:r0 + 64, c,
                                               qb * 512:(qb + 1) * 512],
                                            start=True, stop=True,
                                            tile_position=(r0, 0))
                                    ex = expp.tile([128, GRP * 512], bf16,
                                                   tag="ex", name="ex")
                                    n = len(kts) * 512
                                    nc.scalar.activation(ex[:, :n], sc[:, :n],
                                                         AF.Exp, scale=0.125)
                                    exs.append(ex)
                                if pending_norm is not None:
                                    pending_norm()
                                    pending_norm = None
                                for hh in range(2):
                                    i = 2 * c + hh
                                    ex = exs[hh]
                                    for jj, kt in enumerate(kts):
                                        if kt >= 4 * qb:  # diagonal: causal zero
                                            nc.gpsimd.affine_select(
                                                out=ex[:, jj * 512:(jj + 1) * 512],
                                                in_=ex[:, jj * 512:(jj + 1) * 512],
                                                compare_op=Alu.is_ge,
                                                fill=0.0,
                                                base=512 * qb - 128 * kt,
                                                pattern=[[1, 512]],
                                                channel_multiplier=-1)
                                        nc.tensor.matmul(
                                            zps[hh][:],
                                            V[:, kt, i * (DH + 1):
                                              i * (DH + 1) + 128],
                                            ex[:, jj * 512:(jj + 1) * 512],
                                            start=(kt == 0), stop=(kt == nkt - 1))
                            pending_norm = make_norm(c, qb, zps)
                    if pending_norm is not None:
                        pending_norm()
                        pending_norm = None

                # ------------ Phase D: output projection -------------------
                with tc.tile_pool(name="outsb", bufs=2) as outsb:
                    for qt in range(ST):
                        osb = outsb.tile([128, DM], f32, tag="osb", name="osb")
                        for mb in range(2):
                            po = apsum.tile([128, 512], f32, tag="acc", name="po")
                            for c in range(NH // 2):
                                nc.tensor.matmul(
                                    po, zTf[:, c, qt * 128:(qt + 1) * 128],
                                    woT[:, c, mb * 512:(mb + 1) * 512],
                                    start=(c == 0), stop=(c == NH // 2 - 1))
                            nc.vector.tensor_copy(osb[:, mb * 512:(mb + 1) * 512],
                                                  po)
                        nc.sync.dma_start(out_d.ap()[qt * 128:(qt + 1) * 128, :],
                                          osb[:])

    nc.compile()
    return nc


def _get_nc():
    global _BUILT
    if _BUILT is None:
        _BUILT = _build()
    return _BUILT


def run(inputs, trace=False):
    from concourse import bass_utils

    nc = _get_nc()
    x = np.ascontiguousarray(inputs["x"], dtype=np.float32)
    pos = np.ascontiguousarray(inputs["pos_embed"], dtype=np.float32)
    wq, wk, wv, wo = (np.asarray(inputs[k], dtype=np.float32)
                      for k in ("W_Q", "W_K", "W_V", "W_O"))
    in_maps = []
    for core in range(8):
        b, g = core // 2, core % 2
        hs = slice(g * NH, (g + 1) * NH)
        in_maps.append({
            "x_s": np.ascontiguousarray(x[b]),
            "pos_s": np.ascontiguousarray(pos[b]),
            "wq_s": np.ascontiguousarray(wq[hs]),
            "wk_s": np.ascontiguousarray(wk[hs]),
            "wv_s": np.ascontiguousarray(wv[hs]),
            "wo_s": np.ascontiguousarray(wo[hs]),
        })
    res = bass_utils.run_bass_kernel_spmd(
        nc, in_maps, core_ids=list(range(8)), trace=trace)
    out = np.empty((4, SEQ, DM), dtype=np.float32)
    for b in range(4):
        out[b] = res.results[2 * b]["out_s"] + res.results[2 * b + 1]["out_s"]
    return out, res.exec_time_ns


def kernel(**inputs):
    out, _ = run(inputs, trace=False)
    return out
